# revision 57
# baseline (speedup 1.0000x reference)
"""Trainium2 Bass kernel: gated causal self-attention (GQA + partial RoPE).

Reference computation (per batch):
    q,k,v = x@Wq, x@Wk, x@Wv  (heads split, partial RoPE on first R dims)
    att = softmax(causal(q k^T / sqrt(D)))
    att = att * (att >= sigmoid(gate))          # post-softmax threshold gate
    y = (att @ v) @ Wo

Sharding over 8 NeuronCores: core = 4*b + g where b in {0,1} is the batch
(data parallel) and g in {0..3} is the KV-head group (tensor parallel:
Wq/Wk/Wv column-sharded, Wo row-sharded; gate sharded with heads).  Each
core computes a partial y^T (C x T); the host sums the 4 group partials
per batch and transposes.  The TxT score tensor never leaves a core.

v5: the kernel is PE-stream-bound (~226us of matmul columns at full
clock), so the emission order keeps the PE saturated:
  - h-major scores per t-block with den trailing one head and att@v two
    heads behind, so the ACT exp stream never stalls the PE; "filler"
    matmuls (next block's projections, previous blocks' outproj) are
    pumped between score/den/av groups, with per-head selective
    flushing (PROJ_NEED) instead of a block-boundary barrier;
  - causal diagonal trim: for s-tile i in t-block j, score/exp/mask/
    den/av all skip the t-columns below 128*(i-4j), which are entirely
    above the diagonal (-37k PE columns, -17us ACT);
  - partial RoPE with the rope dims host-permuted into pair-interleaved
    order so rotate_half is one DVE stream_shuffle (adjacent-pair swap)
    instead of two cross-partition ACT copies;
  - startup: weight stream on the ACT DGE queue (ones first as warm-up
    fodder), x blocks on SP, both batched 4-chunks-per-DMA (each DMA
    costs its sequencer ~610ns of issue time); PE p-state is ramped by
    short warm matmuls on `ones`;
  - outproj drains alternate ACT/DVE and stage 4 co-tiles per output
    DMA; final block: co 0-3 pre-accumulate heads 0-2 early, and the
    last task's gating groups interleave with its att@v chunks;
  - causal masks are iota-predicated affine_select ops on the otherwise
    idle GpSimd engine; q/k run in f16 (scores band ~7e-4, comparable
    to the f16 es rounding); gating is 2 2x-mode DVE passes.

PSUM budget (8 banks): sp x3 (score tiles + att@v accumulator), dn x1
(denominator), prj x2 (projection groups), acc x2 (outproj tiles +
v-transposes).
"""

from collections import deque

import numpy as np

import concourse.bass as bass
import concourse.tile as tile
from concourse import bacc, mybir
from concourse.alu_op_type import AluOpType
from concourse.bass_utils import run_bass_kernel_spmd

B, T, C = 2, 2048, 2048
H, HKV, D = 16, 4, 128
R = 64
NCORE = 8
G = 4            # tensor-parallel degree over KV heads
HL = H // G      # 4 local q heads per core
DL = HL * D      # 512 local q dims per core
SCALE = float(D) ** -0.5

F32 = mybir.dt.float32
F32R = mybir.dt.float32r
F16 = mybir.dt.float16
EXP = mybir.ActivationFunctionType.Exp

TB = 512                 # t-block width
NTB = T // TB            # 4
NCT = C // 128           # 16 contraction tiles
GB = 4                   # s-tiles per batched DVE gating op

EYE0, THR0 = 0, 128
CONST_W = 128 + HL


def build():
    nc = bacc.Bacc("TRN2", target_bir_lowering=False, debug=False)

    xT = nc.dram_tensor("xT", [C, T], F16, kind="ExternalInput").ap()
    wq = nc.dram_tensor("wq", [C, DL], F16, kind="ExternalInput").ap()
    wk = nc.dram_tensor("wk", [C, D], F16, kind="ExternalInput").ap()
    wv = nc.dram_tensor("wv", [C, D], F16, kind="ExternalInput").ap()
    wo = nc.dram_tensor("wo", [DL, C], F16, kind="ExternalInput").ap()
    ones = nc.dram_tensor("ones", [128, 128], F16, kind="ExternalInput").ap()
    cs = nc.dram_tensor("cs", [R, T], F16, kind="ExternalInput").ap()
    sn = nc.dram_tensor("sn", [R, T], F16, kind="ExternalInput").ap()
    cst = nc.dram_tensor("cst", [128, CONST_W], F32, kind="ExternalInput").ap()
    ypT = nc.dram_tensor("ypT", [C, T], F16, kind="ExternalOutput").ap()

    with tile.TileContext(nc) as tc:
        with (
            tc.tile_pool(name="persist", bufs=1) as persist,
            tc.tile_pool(name="wpool", bufs=1) as wpool,
            tc.tile_pool(name="xpool", bufs=3) as xpool,
            tc.tile_pool(name="espool", bufs=1) as espool,
            tc.tile_pool(name="qpool", bufs=2) as qpool,
            tc.tile_pool(name="ypool", bufs=2) as ypool,
            tc.tile_pool(name="small", bufs=1) as small,
            tc.tile_pool(name="psum", bufs=1, space="PSUM") as psum,
        ):
            # ---- persistent SBUF ----
            kt = persist.tile([128, T], F16)     # k^T (D x T), rope applied
            vn = persist.tile([128, T], F16)     # v natural; s-tile i at cols 128i
            cs_sb = persist.tile([R, T], F16)    # cos^T
            sn_sb = persist.tile([R, T], F16)    # [-sinT[0:32] ; sinT[32:64]]
            ones_sb = persist.tile([128, 128], F16)
            cst_sb = persist.tile([128, CONST_W], F32)
            eye_sb = cst_sb[:, EYE0 : EYE0 + 128]

            wq_sb = wpool.tile([128, NCT, DL], F16, tag="wq", name="wq_sb")
            wk_sb = wpool.tile([128, NCT, D], F16, tag="wk", name="wk_sb")
            wv_sb = wpool.tile([128, NCT, D], F16, tag="wv", name="wv_sb")
            wo_sb = wpool.tile([128, HL, C], F16, tag="wo", name="wo_sb")
            xss = {0: xpool.tile([128, NCT, TB], F16, tag="xs", name="xs_0")}
            xs = xss[0]

            # Startup DMAs on TWO hwdge queues in parallel (each DMA costs its
            # sequencer ~610ns of issue time, which was the v3/v4 startup
            # bottleneck): ACT carries the weight stream (ones first: it's
            # the warm-up fodder and lands ~1.3us), SP carries the x blocks.
            # wq/wk/wv batched per 4 c-tiles to match proj0's chunk-major
            # consumption; all four x blocks are issued up front (xs_3
            # reuses xs_0's buffer, its DMA legally parks on the WAR sem
            # until proj0's reads finish -- nothing else queues behind it
            # on SP until the first outproj writes ~36us in).
            wkr = wk.rearrange("(i p) f -> p i f", p=128)
            wvr = wv.rearrange("(i p) f -> p i f", p=128)
            wqr = wq.rearrange("(i p) f -> p i f", p=128)
            xTr = xT.rearrange("(i p) f -> p i f", p=128)
            nc.scalar.dma_start(ones_sb[:], ones)
            for c4 in range(0, NCT, 4):
                g = slice(c4, c4 + 4)
                nc.scalar.dma_start(wq_sb[:, g, :], wqr[:, g, :])
                nc.scalar.dma_start(wk_sb[:, g, :], wkr[:, g, :])
                nc.scalar.dma_start(wv_sb[:, g, :], wvr[:, g, :])
            nc.scalar.dma_start(cs_sb[:], cs)
            nc.scalar.dma_start(sn_sb[:], sn)
            nc.scalar.dma_start(cst_sb[:], cst)
            for d in range(HL):
                nc.scalar.dma_start(wo_sb[:, d, :], wo[128 * d : 128 * (d + 1), :])
            for c in range(NCT):
                csl = slice(128 * c, 128 * (c + 1))
                nsp = 4 if c == 0 else 1
                for u in range(nsp):
                    fsl = slice(TB * u // nsp, TB * (u + 1) // nsp)
                    nc.sync.dma_start(xs[:, c, fsl], xT[csl, fsl])
            for pj in (1, 2):
                xs_n = xpool.tile([128, NCT, TB], F16, tag="xs", name=f"xs_{pj}")
                xss[pj] = xs_n
                for c4 in range(0, NCT, 4):
                    nc.sync.dma_start(
                        xs_n[:, c4 : c4 + 4, :],
                        xTr[:, c4 : c4 + 4, TB * pj : TB * (pj + 1)],
                    )

            # PE warm-up: short matmuls on the ones tile (first ACT DMA, so
            # available ~9.5us in after engine bring-up) ramp the PE p-state
            # while proj0's first chunks land
            for w in range(20):
                wp = psum.tile([128, TB], F32, tag="sp", bufs=3, name=f"warm_{w}")
                nc.tensor.matmul(
                    wp[:, 0:128], ones_sb, ones_sb, start=True, stop=True
                )

            es = [
                espool.tile([128, NTB * 4 * TB], F16, tag=f"es{k}", name=f"es{k}")
                for k in range(HL)
            ]

            def es_of(j, h):
                return es[h]
            qtbs = {}
            ytbs = {}

            # ---- PE filler machinery ----
            fillers = deque()  # (kind, idx, closure); closure emits ~4 matmuls

            def pump(n=1):
                for _ in range(n):
                    if fillers:
                        fillers.popleft()[2]()

            def flush_outs_upto(oj_max):
                rest = []
                for it in fillers:
                    if it[0] == "out" and it[1] <= oj_max:
                        it[2]()
                    else:
                        rest.append(it)
                fillers.clear()
                fillers.extend(rest)

            proj_runs = {}

            def flush_proj(j, n):
                """Run fillers (FIFO, any kind) until >= n proj-units of
                block j have executed.  Replaces the all-at-once proj flush,
                whose burst of leftover drains+ropes collided with the
                gating DVE chain at every block boundary."""
                while proj_runs.get(j, 0) < n and fillers:
                    fillers.popleft()[2]()

            # rope dims are host-permuted into pair-interleaved order
            # [0,32,1,33,...]: rotate_half becomes an adjacent-pair swap,
            # which is a single within-quadrant DVE stream_shuffle instead
            # of two ~720ns cross-partition ACT copies (dot products are
            # permutation-invariant, so scores are unchanged)
            SWAP_MASK = [i ^ 1 for i in range(32)]

            def rope(th, dcols, tcols, name):
                """partial RoPE in place on rows 0:R of th[:, dcols] (f16)."""
                rot = small.tile([R, TB], F16, tag="rot", bufs=1, name=f"rot_{name}")
                nc.vector.stream_shuffle(rot[:], th[0:R, dcols], mask=SWAP_MASK)
                nc.vector.tensor_tensor(
                    th[0:R, dcols], th[0:R, dcols], cs_sb[:, tcols], op=AluOpType.mult
                )
                nc.vector.tensor_tensor(
                    rot[:], rot[:], sn_sb[:, tcols], op=AluOpType.mult
                )
                nc.vector.tensor_tensor(
                    th[0:R, dcols], th[0:R, dcols], rot[:], op=AluOpType.add
                )

            def new_qtb(pj):
                qtbs[pj] = qpool.tile([128, HL * TB], F16, tag="qtb", name=f"qtb_{pj}")
                return qtbs[pj]

            def drain_q(gp, qtb, h, tsl, pj):
                dsl = slice(TB * h, TB * (h + 1))
                if h % 2 == 0:
                    nc.scalar.copy(qtb[:, dsl], gp[:])
                else:
                    nc.vector.tensor_copy(qtb[:, dsl], gp[:])
                rope(qtb, dsl, tsl, f"q{pj}_{h}")

            def drain_k(gp, tsl, pj):
                nc.scalar.copy(kt[:, tsl], gp[:])
                rope(kt, tsl, tsl, f"k{pj}")

            def vtrans(vt, pj):
                tp = psum.tile([128, TB], F32, tag="acc", bufs=2, name=f"tp_{pj}")
                for u in range(TB // 128):
                    nc.tensor.transpose(
                        tp[:, 128 * u : 128 * (u + 1)], vt[:, 128 * u : 128 * (u + 1)], eye_sb
                    )
                s0 = pj * 4
                nc.vector.tensor_copy(vn[:, 128 * s0 : 128 * (s0 + 4)], tp[:])

            def make_proj_units(pj, part="all"):
                """Filler units computing q/k/v for block pj from xss[pj]
                (group-major: <=2 'prj' PSUM accumulators live).  Unit order
                [k, q0, q1, v, vtrans, q2, q3] matches flush_proj's needs:
                scores(pj,0) takes 8 units, (pj,1) 12, av(pj,0) 17,
                scores(pj,2) 21, (pj,3) 25.  part="k" emits only the
                k-group: it is the one group that is safe to run two blocks
                early (kt[:, tsl] has no readers before block pj, so its
                drain never parks an in-order engine queue -- q-drains
                would, via the qtb buffer WAR, and that deadlocks against
                the exp stream freeing score PSUM bufs)."""
                tsl = slice(pj * TB, (pj + 1) * TB)
                xs = xss[pj]
                units = []

                def add_unit(fn):
                    def counted(fn=fn):
                        fn()
                        proj_runs[pj] = proj_runs.get(pj, 0) + 1
                    units.append(("proj", pj, counted))

                def group(w_sb, col0, ncols, drain):
                    gp = psum.tile(
                        [128, TB], F32, tag="prj", bufs=2, name=f"prj_{pj}_{col0}_{ncols}"
                    )
                    for cu in range(4):
                        def u(gp=gp, cu=cu, w_sb=w_sb, col0=col0, ncols=ncols, drain=drain):
                            for c in range(4 * cu, 4 * cu + 4):
                                nc.tensor.matmul(
                                    gp[:],
                                    w_sb[:, c, col0 : col0 + ncols],
                                    xs[:, c, :],
                                    start=(c == 0),
                                    stop=(c == NCT - 1),
                                )
                            if cu == 3:
                                drain(gp)
                        add_unit(u)

                if part in ("k", "all"):
                    group(wk_sb, 0, D, lambda gp: drain_k(gp, tsl, pj))
                if part == "k":
                    return units
                qtb = new_qtb(pj)
                for h in range(2):
                    group(wq_sb, 128 * h, 128,
                          lambda gp, h=h: drain_q(gp, qtb, h, tsl, pj))
                vt = small.tile([128, TB], F32, tag="vt", bufs=1, name=f"vt_{pj}")

                def vdrain(gp):
                    nc.scalar.copy(vt[:], gp[:])
                group(wv_sb, 0, D, vdrain)
                add_unit(lambda: vtrans(vt, pj))
                for h in range(2, HL):
                    group(wq_sb, 128 * h, 128,
                          lambda gp, h=h: drain_q(gp, qtb, h, tsl, pj))
                return units

            def emit_proj0():
                """Block-0 projections, chunk-major (matches DMA arrival).
                All PSUM banks are free at startup: qp0-2 on 'sp', qp3 on
                'dn', kp/vp on 'prj'."""
                tsl = slice(0, TB)
                qtb = new_qtb(0)
                qps = [
                    psum.tile([128, TB], F32, tag=("sp" if h < 3 else "dn"),
                              bufs=(3 if h < 3 else 1), name=f"qp0_{h}")
                    for h in range(HL)
                ]
                kp = psum.tile([128, TB], F32, tag="prj", bufs=2, name="kp0")
                vp = psum.tile([128, TB], F32, tag="prj", bufs=2, name="vp0")
                groups = [(qps[h], wq_sb, 128 * h, 128) for h in range(HL)]
                groups += [(kp, wk_sb, 0, D), (vp, wv_sb, 0, D)]
                for ch in range(4):
                    for gp, w_sb, col0, ncols in groups:
                        for ci in range(4):
                            c = 4 * ch + ci
                            nc.tensor.matmul(
                                gp[:], w_sb[:, c, col0 : col0 + ncols], xs[:, c, :],
                                start=(c == 0), stop=(c == NCT - 1),
                            )
                # k first: scores(0,0) needs kt+rope AND qtb[h0]; v last and
                # its PE transpose deferred to a filler (only av(0,0) needs
                # vn, two tasks later) so scores(0,0) starts sooner
                drain_k(kp, tsl, 0)
                for h in range(HL):
                    drain_q(qps[h], qtb, h, tsl, 0)
                vt = small.tile([128, TB], F32, tag="vt", bufs=1, name="vt_0")
                nc.vector.tensor_copy(vt[:], vp[:])
                fillers.append(("out", -2, lambda: vtrans(vt, 0)))

            ypTr = ypT.rearrange("(i p) f -> p i f", p=128)

            def make_outproj_units(oj, tail=False, co_start=0):
                """Output projection for block oj; reads ytbs[oj] (ytb is
                double-buffered, so these may run one block late).  The tail
                variant (after the last att@v) rotates op tiles through the
                now-idle PSUM tags.  Drained co-tiles are staged in groups of
                4 and shipped with ONE dma (SP issues a DMA per ~565ns, so
                per-co DMAs made the final block's write-out SP-bound)."""
                tsl = slice(oj * TB, (oj + 1) * TB)
                ytb = ytbs[oj]
                units = []
                tags = [("acc", 2), ("sp", 3), ("prj", 2)] if tail else [("acc", 2)]
                stg4s = {}
                for co in range(co_start, NCT):
                    def u(co=co):
                        tg, nb = tags[co % len(tags)]
                        op = psum.tile([128, TB], F32, tag=tg, bufs=nb, name=f"op_{oj}_{co}")
                        for d in range(HL):
                            nc.tensor.matmul(
                                op[:],
                                wo_sb[:, d, 128 * co : 128 * (co + 1)],
                                ytb[:, TB * d : TB * (d + 1)],
                                start=(d == 0),
                                stop=(d == HL - 1),
                            )
                        if co % 4 == 0:
                            stg4s[co // 4] = small.tile(
                                [128, 4, TB], F16, tag="stg", bufs=2,
                                name=f"stg_{oj}_{co // 4}",
                            )
                        stg = stg4s[co // 4]
                        # alternate the PSUM drain between ACT and DVE so
                        # neither engine's queue backs up behind exp/gating;
                        # block-2 units are pumped during block 3 where DVE
                        # paces the gating chain, so they drain on ACT
                        if oj == 2 or (co % 2 == 0) != tail:
                            nc.scalar.copy(stg[:, co % 4, :], op[:])
                        else:
                            nc.vector.tensor_copy(stg[:, co % 4, :], op[:])
                        if co % 4 == 3:
                            nc.sync.dma_start(
                                ypTr[:, co - 3 : co + 1, tsl], stg[:]
                            )
                    units.append(("out", oj, u))
                return units

            emit_proj0()

            rdens = {}

            def tile_c0(j, i):
                """First needed t-col (block-local) of s-tile i in t-block j:
                cols below 128*(i-4j) are entirely above the causal diagonal.
                Scores/exp/mask/den/av all skip [0, c0); the skipped es cols
                hold stale/uninit data that is never read."""
                di = i - 4 * j
                return 128 * di if di > 0 else 0

            def emit_scores(j, h):
                qtb = qtbs[j]
                nst = 4 * j + 4
                est_all = es_of(j, h)
                for i in range(nst):
                    c0 = tile_c0(j, i)
                    sp = psum.tile([128, TB], F32, tag="sp", bufs=3, name=f"sp_{j}_{h}_{i}")
                    nc.tensor.matmul(
                        sp[:, c0:TB], kt[:, 128 * i : 128 * (i + 1)],
                        qtb[:, TB * h + c0 : TB * (h + 1)],
                        start=True, stop=True,
                    )
                    est = est_all[:, TB * i + c0 : TB * (i + 1)]
                    nc.scalar.activation(est, sp[:, c0:TB], EXP, scale=SCALE)
                    if i >= 4 * j:
                        # causal: keep where t_glob >= s_glob; with the c0
                        # trim this is f_local >= p for every diagonal tile
                        nc.gpsimd.affine_select(
                            out=est,
                            in_=est,
                            pattern=[[1, TB - c0]],
                            compare_op=AluOpType.is_ge,
                            fill=0.0,
                            base=0,
                            channel_multiplier=-1,
                        )
                    if i % 2 == 1:
                        pump(1)

            cthrs = {}

            def gate_group(j, h, g0):
                nst = 4 * j + 4
                gn = min(GB, nst - g0)
                cthr = cthrs[(j, h)]
                ev = es_of(j, h)[:, TB * g0 : TB * (g0 + gn)].rearrange(
                    "p (r n) -> p r n", r=gn
                )
                cb = cthr[:][:, None, :].broadcast_to([128, gn, TB])
                msk = small.tile(
                    [128, GB * TB], F16, tag="msk", bufs=1, name=f"mk_{j}_{h}_{g0}"
                )
                mv = msk[:, 0 : TB * gn].rearrange("p (r n) -> p r n", r=gn)
                nc.vector.tensor_tensor(mv, ev, cb, op=AluOpType.is_ge)
                nc.vector.tensor_tensor(ev, ev, mv, op=AluOpType.mult)

            def emit_den(j, h, gate=True):
                nst = 4 * j + 4
                est_all = es_of(j, h)
                pump(2)
                dn = psum.tile([128, TB], F32, tag="dn", bufs=1, name=f"dn_{j}_{h}")
                for i in range(nst):
                    c0 = tile_c0(j, i)
                    nc.tensor.matmul(
                        dn[:, c0:TB], ones_sb, est_all[:, TB * i + c0 : TB * (i + 1)],
                        start=(i == 0), stop=(i == nst - 1),
                    )
                    if i % 4 == 3:
                        pump(1)
                cthr = small.tile([128, TB], F16, tag="cthr", bufs=1, name=f"ct_{j}_{h}")
                cthrs[(j, h)] = cthr
                rden = small.tile([128, TB], F32, tag="rden", bufs=3, name=f"rd_{j}_{h}")
                rdens[(j, h)] = rden
                nc.vector.tensor_scalar_mul(
                    cthr[:], dn[:], cst_sb[:, THR0 + h : THR0 + h + 1]
                )
                nc.vector.reciprocal_approx_fast(out=rden[:], in_=dn[:])
                if gate:
                    for g0 in range(0, nst, GB):
                        gate_group(j, h, g0)

            def av_chunk(j, h, yp, i0, i1):
                nst = 4 * j + 4
                est_all = es_of(j, h)
                for i in range(i0, i1):
                    c0 = tile_c0(j, i)
                    nc.tensor.matmul(
                        yp[:, c0:TB], vn[:, 128 * i : 128 * (i + 1)],
                        est_all[:, TB * i + c0 : TB * (i + 1)],
                        start=(i == 0), stop=(i == nst - 1),
                    )
                    if i % 4 == 3:
                        pump(1)

            def av_norm(j, h, yp):
                nc.vector.tensor_tensor(
                    ytbs[j][:, TB * h : TB * (h + 1)], yp[:], rdens[(j, h)][:],
                    op=AluOpType.mult,
                )

            def emit_av(j, h):
                nst = 4 * j + 4
                pump(4)
                yp = psum.tile([128, TB], F32, tag="sp", bufs=3, name=f"yp_{j}_{h}")
                av_chunk(j, h, yp, 0, nst)
                av_norm(j, h, yp)

            # flat task stream: scores at t, den trails 1 task, av trails 2;
            # block boundaries only gate projections/buffers, not the
            # exp/gating pipelines
            NT = 4 * NTB

            PROJ_NEED = (8, 12, 21, 25)
            early_k = set()

            def on_task(t):
                j, h = t // 4, t % 4
                if h == 0:
                    ytbs[j] = ypool.tile([128, HL * TB], F16, tag="ytb", name=f"ytb_{j}")
                    if j + 2 < NTB and (j + 2) not in xss:
                        xs_n = xpool.tile([128, NCT, TB], F16, tag="xs", name=f"xs_{j+2}")
                        xss[j + 2] = xs_n
                        tc0 = (j + 2) * TB
                        for c4 in range(0, NCT, 4):
                            nc.sync.dma_start(
                                xs_n[:, c4 : c4 + 4, :],
                                xTr[:, c4 : c4 + 4, tc0 : tc0 + TB],
                            )
                    if j + 1 < NTB:
                        part = "rest" if (j + 1) in early_k else "all"
                        fillers.extend(make_proj_units(j + 1, part))
                if h == 2 and j + 2 < NTB:
                    # the k-group of block j+2 is safe two blocks early and
                    # fills the PE starvation at the next block boundary
                    fillers.extend(make_proj_units(j + 2, "k"))
                    early_k.add(j + 2)
                if j > 0:
                    flush_proj(j, PROJ_NEED[h])
                emit_scores(j, h)

            reserve12, reserve15, reserve_last = [], [], []
            for t in range(NT):
                if t == 4 * (NTB - 1):
                    fillers.extend(reserve12)
                    reserve12.clear()
                on_task(t)
                if t == NT - 1:
                    fillers.extend(reserve15)
                    reserve15.clear()
                if t >= 1:
                    emit_den((t - 1) // 4, (t - 1) % 4)
                if t >= 2:
                    ta = t - 2
                    ja, ha = ta // 4, ta % 4
                    if ha == 0:
                        # av writes ytb buffer ja%2: everything still reading
                        # ytb_{ja-2} (same buffer) must be emitted first
                        flush_outs_upto(ja - 2)
                    emit_av(ja, ha)
                    if ha == HL - 1:
                        units = make_outproj_units(ja)
                        if ja == NTB - 3:
                            # release half at block 3's scores phase
                            reserve12.extend(units[8:])
                            units = units[:8]
                        elif ja == NTB - 2:
                            # hold all for block 3's av phase, where the
                            # DVE gating chain starves the PE; the last few
                            # cover the final att@v's gating wait
                            reserve15.extend(units[:8])
                            reserve_last.extend(units[8:])
                            units = []
                        fillers.extend(units)

            emit_den(NTB - 1, HL - 1, gate=False)
            emit_av(NTB - 1, HL - 2)
            fillers.extend(reserve_last)
            reserve_last.clear()
            pump(4)
            # final task: interleave gating groups with att@v chunks so the
            # PE starts att@v as soon as the first gated tiles are ready,
            # and pre-accumulate heads 0-2 of outproj co 0-3 (only ytb_3[h3]
            # is missing) to fill the PE while the DVE gates
            j3, h3 = NTB - 1, HL - 1
            nst3 = 4 * j3 + 4
            ytb3 = ytbs[j3]
            tsl3 = slice(j3 * TB, (j3 + 1) * TB)
            yp3 = psum.tile([128, TB], F32, tag="sp", bufs=3, name="yp_3_3")
            early_tags = [("prj", 2), ("prj", 2), ("sp", 3), ("sp", 3)]
            early_ops = []
            for co in range(4):
                tg, nb = early_tags[co]
                op = psum.tile([128, TB], F32, tag=tg, bufs=nb, name=f"opE_{co}")
                early_ops.append(op)
                for d in range(HL - 1):
                    nc.tensor.matmul(
                        op[:], wo_sb[:, d, 128 * co : 128 * (co + 1)],
                        ytb3[:, TB * d : TB * (d + 1)],
                        start=(d == 0), stop=False,
                    )
            for g0 in range(0, nst3, GB):
                gate_group(j3, h3, g0)
                pump(2)
                av_chunk(j3, h3, yp3, g0, min(g0 + GB, nst3))
            av_norm(j3, h3, yp3)
            stgE = small.tile([128, 4, TB], F16, tag="stg", bufs=2, name="stg_3_E")
            for co in range(4):
                nc.tensor.matmul(
                    early_ops[co][:], wo_sb[:, HL - 1, 128 * co : 128 * (co + 1)],
                    ytb3[:, TB * (HL - 1) : TB * HL],
                    start=False, stop=True,
                )
                if co % 2 == 0:
                    nc.scalar.copy(stgE[:, co, :], early_ops[co][:])
                else:
                    nc.vector.tensor_copy(stgE[:, co, :], early_ops[co][:])
            nc.sync.dma_start(ypTr[:, 0:4, tsl3], stgE[:])
            fillers.extend(make_outproj_units(NTB - 1, tail=True, co_start=4))
            while fillers:
                fillers.popleft()[2]()

    nc.compile()
    return nc


_NC_CACHE = None


def _get_nc():
    global _NC_CACHE
    if _NC_CACHE is None:
        _NC_CACHE = build()
    return _NC_CACHE


def make_in_maps(x, cos, sin, Wq, Wk, Wv, Wo, gate):
    x = np.asarray(x, np.float32)
    cos = np.asarray(cos, np.float32)
    sin = np.asarray(sin, np.float32)
    Wq = np.asarray(Wq, np.float32)
    Wk = np.asarray(Wk, np.float32)
    Wv = np.asarray(Wv, np.float32)
    Wo = np.asarray(Wo, np.float32)
    gate = np.asarray(gate, np.float32)

    hw = R // 2
    # pair-interleave the rope dims ([0,32,1,33,...]) so rotate_half is an
    # adjacent-pair swap on-device (DVE stream_shuffle); permute the rope
    # tables and the first-R output columns of every head block of Wq/Wk
    # identically -- q.k dot products are invariant to the permutation
    perm = np.empty(R, np.int64)
    perm[0::2] = np.arange(hw)
    perm[1::2] = np.arange(hw, R)
    cosT = np.ascontiguousarray(cos.T[perm]).astype(np.float16)  # (R, T)
    sinT = sin.T
    sn_signed = np.ascontiguousarray(
        np.concatenate([-sinT[0:hw], sinT[hw:R]], axis=0)[perm]
    ).astype(np.float16)
    Wq = Wq.copy()
    Wk = Wk.copy()
    for hh in range(H):
        Wq[:, D * hh : D * hh + R] = Wq[:, D * hh : D * hh + R][:, perm]
    for kk in range(HKV):
        Wk[:, D * kk : D * kk + R] = Wk[:, D * kk : D * kk + R][:, perm]
    thr_full = 1.0 / (1.0 + np.exp(-gate))  # sigmoid, (H,)
    cst_base = np.zeros((128, CONST_W), np.float32)
    cst_base[:, EYE0 : EYE0 + 128] = np.eye(128, dtype=np.float32)
    ones16 = np.ones((128, 128), np.float16)

    in_maps = []
    for core in range(NCORE):
        b, g = divmod(core, G)
        cst = cst_base.copy()
        cst[:, THR0 : THR0 + HL] = thr_full[HL * g : HL * (g + 1)]
        in_maps.append(
            {
                "xT": np.ascontiguousarray(x[b].T).astype(np.float16),
                "wq": np.ascontiguousarray(Wq[:, DL * g : DL * (g + 1)]).astype(np.float16),
                "wk": np.ascontiguousarray(Wk[:, D * g : D * (g + 1)]).astype(np.float16),
                "wv": np.ascontiguousarray(Wv[:, D * g : D * (g + 1)]).astype(np.float16),
                "wo": np.ascontiguousarray(Wo[DL * g : DL * (g + 1), :].astype(np.float16)),
                "ones": ones16,
                "cs": cosT,
                "sn": sn_signed,
                "cst": cst,
            }
        )
    return in_maps


def run(inputs, trace=False, **kw):
    """Run on 8 NeuronCores; returns (y_full, BassKernelResults)."""
    nc = _get_nc()
    in_maps = make_in_maps(**inputs)
    res = run_bass_kernel_spmd(nc, in_maps, core_ids=list(range(NCORE)), trace=trace, **kw)
    y = np.zeros((B, T, C), np.float32)
    for core in range(NCORE):
        b = core // G
        y[b] += res.results[core]["ypT"].T.astype(np.float32)
    return y, res


def kernel(**inputs) -> np.ndarray:
    y, _ = run(inputs)
    return y

